# revision 19
# baseline (speedup 1.0000x reference)
"""Trainium2 Bass kernel for nn_DisAttLayer (disentangled-attention bias MLP).

Math (reference):
    e[b,m,n,h,:] = concat(pe[m-n+S], bi[b,m], bj[b,n], ci[b,m], cj[b,n])  (96)
    h1 = relu(e @ w1[:, :, h])     (96->32, per head)
    h2 = relu(h1 @ w2[:, :, h])    (32->16)
    score[b,h,m,n] = h2 @ w3[:, h] (16->1)

Key factorization: layer 1 is linear in the concat, so
    h1pre[b,m,n,h,k] = Ap[m-n+S,h,k] + Arow[b,m,h,k] + Acol[b,n,h,k]
where Ap/Arow/Acol are tiny per-table transforms (computed on-device from the
raw embedding tables and w1).  With the free axis taken as n' = 255-n, the
relative-position gather Ap[m-n+S] becomes a contiguous slice of a 384-wide
table, so no gather is needed at all.  Only layers 2+3 touch the full
(B,S,S,H) volume.

v2 engine plan (per core):
  - DVE: sliding-window add tmp = Ap[m+n'] + Acol (tensor_tensor, 2x mode)
    plus a share of the per-m bias+relu tensor_scalar ops (4x mode).
  - ACT: h2 = relu(ps2) PSUM evacuation + a share of per-m bias+relu.
  - GPSIMD: a share of per-m bias+relu + the fp16 score evacuation.
  - PE: two persistent stationaries (w2f for stage 2, w3stack for stage 3),
    column-tiled so stage-2's two 64-wide group matmuls and stage-3's four
    32-wide variant matmuls run concurrently in disjoint column groups.
  - One fat [128, 512] output DMA per m-block (16 total).

Sharding: 8 cores = batch b (4) x query-half m (2), single SPMD program.
Host does layout only (transpose/reshape/zero-pad/one-hot relabeling).
"""

import os
from contextlib import ExitStack

import numpy as np

import concourse.bacc as bacc
import concourse.bass as bass
import concourse.tile as tile
from concourse import mybir
from concourse.bass_utils import run_bass_kernel_spmd

S = 256
H = 8
B = 4
MH = 128          # m-values per core
VB = 11           # e_bi / e_bj rows  (N_MB + 1)
VC = 102          # e_ci / e_cj rows  (N_C + 2)
APW = MH + S      # 384: width of the per-core shifted e_pos slice
NIT = MH // 8     # 16 m-blocks of 8

F32 = mybir.dt.float32
F16 = mybir.dt.float16
BF16 = mybir.dt.bfloat16

# per-it routing of the 16 per-m bias+relu ops (index s = 8*g + j).
# 'd' = DVE tensor_scalar (fast), 'a' = ACT activation, 'g' = GPSIMD.
TS_ROUTE = os.environ.get("BASSK_TS_ROUTE", "ddaddaddadadadaa")
H2_ROUTE = os.environ.get("BASSK_H2_ROUTE", "aa")      # per half: a/d/g
SC_ROUTE = os.environ.get("BASSK_SC_ROUTE", "d")       # score evac: g/d/a
N_WARMUP_MM = int(os.environ.get("BASSK_WARMUP", "10"))
NO_TILEPOS = bool(os.environ.get("BASSK_NO_TILEPOS"))  # drop tile_position args
S3_MODE = os.environ.get("BASSK_S3", "tile")           # stage3: tile | acc
TT_ROUTE = os.environ.get("BASSK_TT_ROUTE", "dd")      # per (it%len, g): d/g
TT_MERGE = os.environ.get("BASSK_TT_MERGE", "0") == "1"  # one [128,4096] TT/it


def _declare_io(nc):
    def inp(name, shape):
        return nc.dram_tensor(name, list(shape), F16, kind="ExternalInput").ap()

    ins = {
        "eposT": inp("eposT", (2, 128, APW)),       # [g][hh*32+d, r_local]
        "ebiT": inp("ebiT", (128, VB)),             # [h*16+d, v]
        "ebjT": inp("ebjT", (128, VB)),
        "eciT": inp("eciT", (128, VC)),
        "ecjT": inp("ecjT", (128, VC)),
        "w1pe_blk": inp("w1pe_blk", (2, 128, 128)),  # [g][hh*32+d, hh*32+k]
        "w1bi_blk": inp("w1bi_blk", (128, 256)),     # [h*16+d, h*32+k]
        "w1bj_blk": inp("w1bj_blk", (128, 256)),
        "w1ci_blk": inp("w1ci_blk", (128, 256)),
        "w1cj_blk": inp("w1cj_blk", (128, 256)),
        "oh_b_row": inp("oh_b_row", (VB, MH)),
        "oh_c_row": inp("oh_c_row", (VC, MH)),
        "oh_b_col": inp("oh_b_col", (VB, S)),        # n-reversed
        "oh_c_col": inp("oh_c_col", (VC, S)),
        "w2blk": inp("w2blk", (2, 128, 64)),         # [g][hh*32+k, hh*16+l]
        # stage-3 stationary: col 32q+h = w3[:, h] at rows [64g+16hh+l]
        "w3stack": inp("w3stack", (128, 128)),
        # zero-padded fallback variants (baseline accumulate scheme)
        "w3blk4": inp("w3blk4", (128, 512)),
    }
    # raw per-it PSUM dumps; host unpacks (rows 32q+h, h<8 are live)
    out = nc.dram_tensor("score_part", [NIT, 128, 512], F16,
                         kind="ExternalOutput").ap()
    return ins, out


def _emit(tc: tile.TileContext, X, out):
    nc = tc.nc
    AL = mybir.AluOpType
    AF = mybir.ActivationFunctionType

    with ExitStack() as ctx:
        const = ctx.enter_context(tc.tile_pool(name="const", bufs=1))
        tabs = ctx.enter_context(tc.tile_pool(name="tabs", bufs=1))
        psum_pre = ctx.enter_context(tc.tile_pool(name="psum_pre", bufs=2, space="PSUM"))

        # ---- load raw inputs to SBUF (fp16, two HWDGE issue queues) ----
        ld_n = [0]

        def load(name, src=None):
            if src is None:
                src = X[name]
            t = const.tile(list(src.shape), F16, name=f"sb_{name}")
            eng = nc.sync if ld_n[0] % 2 == 0 else nc.scalar
            ld_n[0] += 1
            eng.dma_start(out=t, in_=src)
            return t

        # ---- PE warm-up: dummy matmuls so the HAM clock-gate opens before
        # the precompute/main matmuls start; overlaps the input-DMA phase.
        warm_w = const.tile([128, 128], BF16, name="warm_w")
        warm_r = const.tile([128, 512], BF16, name="warm_r")
        nc.vector.memset(warm_w, 0.0)
        nc.vector.memset(warm_r, 0.0)
        ps_warm = psum_pre.tile([128, 512], F32, name="ps_warm", tag="pre")
        for _ in range(N_WARMUP_MM):
            nc.tensor.matmul(out=ps_warm, lhsT=warm_w, rhs=warm_r,
                             start=True, stop=True)

        # critical-path loads first: Ap chain, then Acol chain, then the rest
        eposT = [load(f"eposT{g}", X["eposT"][g]) for g in range(2)]
        w1pe = [load(f"w1pe{g}", X["w1pe_blk"][g]) for g in range(2)]
        ebjT = load("ebjT")
        ecjT = load("ecjT")
        w1bj = load("w1bj_blk")
        w1cj = load("w1cj_blk")
        ohbc = load("oh_b_col")
        ohcc = load("oh_c_col")
        ebiT = load("ebiT")
        eciT = load("eciT")
        w1bi = load("w1bi_blk")
        w1ci = load("w1ci_blk")
        ohbr = load("oh_b_row")
        ohcr = load("oh_c_row")
        w2h = [load(f"w2h_{g}", X["w2blk"][g]) for g in range(2)]
        w2f = tabs.tile([128, 128], F16, name="w2f")
        for g in range(2):
            nc.vector.tensor_copy(w2f[:, 64 * g:64 * (g + 1)], w2h[g])
        w3stack = load("w3stack")
        w3f = load("w3blk4") if S3_MODE == "acc" else None

        # ---- A-tables (all fp32 matmuls, accumulated in PSUM) ----
        # ApM: both groups side by side; col j <-> r_local = j-2 within a group
        ApM = tabs.tile([128, 2 * (APW + 4)], F16, name="ApM")
        Ap = [ApM[:, (APW + 4) * g:(APW + 4) * (g + 1)] for g in range(2)]
        for g in range(2):
            ps = psum_pre.tile([128, APW - 1], F32, name=f"ps_ap{g}", tag="pre")
            nc.tensor.matmul(out=ps, lhsT=w1pe[g], rhs=eposT[g][:, 1:APW],
                             start=True, stop=True)
            nc.vector.tensor_copy(Ap[g][:, 2:APW + 1], ps)

        # T-tables: [v, (h,k)=256] fp16
        T = {}
        for nm, eT, wblk, P in (("bi", ebiT, w1bi, VB), ("ci", eciT, w1ci, VC),
                                ("bj", ebjT, w1bj, VB), ("cj", ecjT, w1cj, VC)):
            ps = psum_pre.tile([P, 256], F32, name=f"ps_t{nm}", tag="pre")
            nc.tensor.matmul(out=ps, lhsT=eT, rhs=wblk, start=True, stop=True)
            t = tabs.tile([P, 256], F16, name=f"T{nm}")
            nc.vector.tensor_copy(t, ps)
            T[nm] = t

        # Arow[g]: [128=(hh,k), MH] fp32 ; AcolM8: [128, 2*8*S] fp16 (x8 copies per g)
        Arow = [tabs.tile([128, MH], F32, name=f"Arow{g}") for g in range(2)]
        AcolM8 = tabs.tile([128, 2 * 8 * S], F16, name="AcolM8")
        Acol8 = [AcolM8[:, 8 * S * g:8 * S * (g + 1)] for g in range(2)]
        for g in range(2):
            cs = slice(128 * g, 128 * (g + 1))
            ps = psum_pre.tile([128, MH], F32, name=f"ps_arow{g}", tag="pre")
            nc.tensor.matmul(out=ps, lhsT=T["bi"][:, cs], rhs=ohbr,
                             start=True, stop=False)
            nc.tensor.matmul(out=ps, lhsT=T["ci"][:, cs], rhs=ohcr,
                             start=False, stop=True)
            nc.vector.tensor_copy(Arow[g], ps)

            ps2 = psum_pre.tile([128, S], F32, name=f"ps_acol{g}", tag="pre")
            nc.tensor.matmul(out=ps2, lhsT=T["bj"][:, cs], rhs=ohbc,
                             start=True, stop=False)
            nc.tensor.matmul(out=ps2, lhsT=T["cj"][:, cs], rhs=ohcc,
                             start=False, stop=True)
            nc.vector.tensor_copy(Acol8[g][:, 0:S], ps2)
            for w in (S, 2 * S, 4 * S):
                nc.vector.tensor_copy(Acol8[g][:, w:2 * w], Acol8[g][:, 0:w])

        # ---- main loop: 16 m-blocks x 8 m-values ----
        work = ctx.enter_context(tc.tile_pool(name="work", bufs=3))
        psum_m = ctx.enter_context(tc.tile_pool(name="psum_m", bufs=1, space="PSUM"))
        psum_o = ctx.enter_context(tc.tile_pool(name="psum_o", bufs=2, space="PSUM"))

        def bias_relu(dst, src, bias_col, eng):
            if eng == "d":
                nc.vector.tensor_scalar(dst, src, bias_col, 0.0, AL.add, AL.max)
            elif eng == "g":
                nc.gpsimd.tensor_scalar(dst, src, bias_col, 0.0, AL.add, AL.max)
            else:
                nc.scalar.activation(out=dst, in_=src, func=AF.Relu,
                                     bias=bias_col, scale=1.0)

        for it in range(NIT):
            m0 = 8 * it
            # -- stage 1a: tmp[g] = Ap[g][:, m+n'+2] + Acol (sliding window)
            if TT_MERGE:
                ap0 = Ap[0]
                apwin2 = bass.AP(
                    ap0.tensor, ap0.offset + (m0 + 2),
                    [list(ap0.ap[0]), [APW + 4, 2], [1, 8], [1, S]],
                )
                tmpM = work.tile([128, 4096], F16, name=f"tmpM_{it}", tag="tmpM")
                if TT_ROUTE[it % len(TT_ROUTE)] == "g":
                    nc.gpsimd.tensor_add(tmpM, apwin2, AcolM8)
                else:
                    nc.vector.tensor_add(tmpM, apwin2, AcolM8)
                tmps = [tmpM[:, 0:2048], tmpM[:, 2048:4096]]
            else:
                tmps = []
                for g in range(2):
                    apg = Ap[g]
                    apwin = bass.AP(
                        apg.tensor, apg.offset + (m0 + 2),
                        [list(apg.ap[0]), [1, 8], [1, S]],
                    )
                    tmp = work.tile([128, 2048], F16, name=f"tmp{g}_{it}",
                                    tag=f"tmp{g}")
                    if TT_ROUTE[(2 * it + g) % len(TT_ROUTE)] == "g":
                        nc.gpsimd.tensor_add(tmp, apwin, Acol8[g])
                    else:
                        nc.vector.tensor_add(tmp, apwin, Acol8[g])
                    tmps.append(tmp)

            # -- stage 1b: h1 = relu(tmp + Arow[:, m]) per m, routed 3 ways
            h1 = [[work.tile([128, 1024], F16, name=f"h1_{g}_{it}_{hf}",
                             tag=f"h1{g}{hf}") for hf in range(2)]
                  for g in range(2)]
            for g in range(2):
                for j in range(8):
                    hf, jj = j // 4, j % 4
                    bias_relu(h1[g][hf][:, S * jj:S * (jj + 1)],
                              tmps[g][:, S * j:S * (j + 1)],
                              Arow[g][:, m0 + j:m0 + j + 1],
                              TS_ROUTE[(8 * g + j) % len(TS_ROUTE)])

            # -- stage 2: both halves, two col-tiled group matmuls each
            ps2h = []
            for hf in range(2):
                ps2 = psum_m.tile([128, 1024], F32, name=f"ps2_{it}_{hf}",
                                  tag=f"ps2{hf}")
                for c in range(2):
                    for g in range(2):
                        nc.tensor.matmul(
                            out=ps2[64 * g:64 * (g + 1), 512 * c:512 * (c + 1)],
                            lhsT=w2f[:, 64 * g:64 * (g + 1)],
                            rhs=h1[g][hf][:, 512 * c:512 * (c + 1)],
                            start=True, stop=True,
                            tile_position=None if NO_TILEPOS else (0, 64 * g),
                        )
                ps2h.append(ps2)

            # -- h2 = relu(ps2): PSUM -> SBUF fp16
            h2 = []
            for hf in range(2):
                t = work.tile([128, 1024], F16, name=f"h2_{it}_{hf}", tag=f"h2{hf}")
                r = H2_ROUTE[hf % len(H2_ROUTE)]
                if r == "a":
                    nc.scalar.activation(out=t, in_=ps2h[hf], func=AF.Relu)
                elif r == "d":
                    nc.vector.tensor_scalar(t, ps2h[hf], 0.0, 0.0, AL.max, AL.bypass)
                else:
                    nc.gpsimd.tensor_scalar(t, ps2h[hf], 0.0, 0.0, AL.max, AL.bypass)
                h2.append(t)

            # -- stage 3: four col-tiled variant matmuls into one PSUM bank
            ps3 = psum_o.tile([128, 512], F32, name=f"ps3_{it}", tag="ps3")
            for hf in range(2):
                for c in range(2):
                    q = 2 * hf + c
                    if S3_MODE == "acc":
                        nc.tensor.matmul(
                            out=ps3, lhsT=w3f[:, 128 * q:128 * (q + 1)],
                            rhs=h2[hf][:, 512 * c:512 * (c + 1)],
                            start=(q == 0), stop=(q == 3),
                        )
                    else:
                        nc.tensor.matmul(
                            out=ps3[32 * q:32 * (q + 1), :],
                            lhsT=w3stack[:, 32 * q:32 * (q + 1)],
                            rhs=h2[hf][:, 512 * c:512 * (c + 1)],
                            start=True, stop=True,
                            tile_position=None if NO_TILEPOS else (0, 32 * q),
                        )

            # -- score evac (fp16) + one fat DMA out
            sc = work.tile([128, 512], F16, name=f"sc_{it}", tag="sc")
            if SC_ROUTE == "g":
                nc.gpsimd.tensor_copy(sc, ps3)
            elif SC_ROUTE == "d":
                nc.vector.tensor_copy(sc, ps3)
            else:
                nc.scalar.copy(sc, ps3)
            nc.sync.dma_start(out=out[it], in_=sc)


_PROGRAM = None


def _get_program():
    global _PROGRAM
    if _PROGRAM is None:
        nc = bacc.Bacc("TRN2", debug=False, num_devices=8)
        ins, out = _declare_io(nc)
        with tile.TileContext(nc) as tc:
            _emit(tc, ins, out)
        nc.compile()
        _PROGRAM = nc
    return _PROGRAM


def _build_in_maps(inputs):
    b_seq = np.asarray(inputs["b_seq"]).astype(np.int64)
    c_seq = np.asarray(inputs["c_seq"]).astype(np.int64)
    e_pos = np.asarray(inputs["e_pos"]).astype(np.float32)   # (512, 8, 32)
    e_bi = np.asarray(inputs["e_bi"]).astype(np.float32)     # (11, 8, 16)
    e_bj = np.asarray(inputs["e_bj"]).astype(np.float32)
    e_ci = np.asarray(inputs["e_ci"]).astype(np.float32)     # (102, 8, 16)
    e_cj = np.asarray(inputs["e_cj"]).astype(np.float32)
    w1 = np.asarray(inputs["w1_e"]).astype(np.float32)       # (96, 32, 8)
    w2 = np.asarray(inputs["w2_e"]).astype(np.float32)       # (32, 16, 8)
    w3 = np.asarray(inputs["w3_e"]).astype(np.float32)       # (16, 8)

    C = lambda a: np.ascontiguousarray(a.astype(np.float16))

    # [h*32+d, r] and [h*16+d, v] transposed table layouts
    eposT_full = C(e_pos.transpose(1, 2, 0).reshape(H * 32, 2 * S))
    ebiT = C(e_bi.transpose(1, 2, 0).reshape(128, VB))
    ebjT = C(e_bj.transpose(1, 2, 0).reshape(128, VB))
    eciT = C(e_ci.transpose(1, 2, 0).reshape(128, VC))
    ecjT = C(e_cj.transpose(1, 2, 0).reshape(128, VC))

    # block-diagonal w1 pieces
    w1pe_blk = np.zeros((2, 128, 128), np.float16)
    for g in range(2):
        for hh in range(4):
            w1pe_blk[g, 32 * hh:32 * (hh + 1), 32 * hh:32 * (hh + 1)] = w1[0:32, :, 4 * g + hh]

    def blk16(w1rows):  # (16, 32, 8) -> [h*16+d, h*32+k]
        m = np.zeros((128, 256), np.float16)
        for h in range(H):
            m[16 * h:16 * (h + 1), 32 * h:32 * (h + 1)] = w1rows[:, :, h]
        return m

    w1bi_blk = blk16(w1[32:48])
    w1bj_blk = blk16(w1[48:64])
    w1ci_blk = blk16(w1[64:80])
    w1cj_blk = blk16(w1[80:96])

    w2blk = np.zeros((2, 128, 64), np.float16)
    for g in range(2):
        for hh in range(4):
            w2blk[g, 32 * hh:32 * (hh + 1), 16 * hh:16 * (hh + 1)] = w2[:, :, 4 * g + hh]

    # stage-3 stationary: rows (g,hh,l) = h2 partition layout; col 32q+h
    w3stack = np.zeros((128, 128), np.float16)
    w3blk4 = np.zeros((128, 512), np.float16)
    for q in range(4):
        for h in range(H):
            w3stack[16 * h:16 * (h + 1), 32 * q + h] = w3[:, h]
            w3blk4[16 * h:16 * (h + 1), 128 * q + 32 * q + h] = w3[:, h]

    shared = {
        "ebiT": ebiT, "ebjT": ebjT, "eciT": eciT, "ecjT": ecjT,
        "w1pe_blk": w1pe_blk, "w1bi_blk": w1bi_blk, "w1bj_blk": w1bj_blk,
        "w1ci_blk": w1ci_blk, "w1cj_blk": w1cj_blk,
        "w2blk": w2blk, "w3stack": w3stack, "w3blk4": w3blk4,
    }

    def onehot(seq_slice, nv):
        oh = np.zeros((nv, len(seq_slice)), np.float16)
        oh[seq_slice, np.arange(len(seq_slice))] = 1.0
        return oh

    in_maps = []
    for core in range(8):
        b, half = core // 2, core % 2
        m_off = half * MH
        im = dict(shared)
        im["eposT"] = C(eposT_full[:, m_off:m_off + APW].reshape(2, 128, APW))
        im["oh_b_row"] = onehot(b_seq[b, m_off:m_off + MH], VB)
        im["oh_c_row"] = onehot(c_seq[b, m_off:m_off + MH], VC)
        im["oh_b_col"] = onehot(b_seq[b, ::-1], VB)
        im["oh_c_col"] = onehot(c_seq[b, ::-1], VC)
        in_maps.append(im)
    return in_maps


def _unpack_part(part):
    """[NIT, 128, 512] fp16 raw dump -> [H, MH, S] fp32 (n-unreversed)."""
    arr = np.asarray(part, np.float32).reshape(NIT, 4, 32, 2, S)
    arr = arr[:, :, :H]                      # live head rows    [it,q,h,u,n']
    arr = arr.transpose(2, 0, 1, 3, 4)       # [h, it, q, u, n']
    arr = arr.reshape(H, MH, S)[:, :, ::-1]  # m = 8*it + 2*q + u; un-reverse n
    return arr


def _assemble(core_outs):
    score = np.empty((B, H, S, S), np.float32)
    for core in range(8):
        b, half = core // 2, core % 2
        score[b, :, half * MH:(half + 1) * MH, :] = _unpack_part(
            core_outs[core]["score_part"])
    return score


def kernel(**inputs) -> np.ndarray:
    in_maps = _build_in_maps(inputs)
    nc = _get_program()

    if os.environ.get("BASSK_SIM"):
        from concourse.bass_interp import CoreSim
        score = np.zeros((B, H, S, S), np.float32)
        for core in [int(x) for x in os.environ["BASSK_SIM"].split(",")]:
            sim = CoreSim(nc, trace=False)
            for k, v in in_maps[core].items():
                sim.tensor(k)[:] = v
            sim.simulate(check_with_hw=False)
            b, half = core // 2, core % 2
            score[b, :, half * MH:(half + 1) * MH, :] = _unpack_part(
                sim.tensor("score_part").copy())
        return score

    res = run_bass_kernel_spmd(nc, in_maps, core_ids=list(range(8)))
    return _assemble(res.results)


# revision 20
# speedup vs baseline: 1.2126x; 1.2126x over previous
"""Trainium2 Bass kernel for nn_DisAttLayer (disentangled-attention bias MLP).

Math (reference):
    e[b,m,n,h,:] = concat(pe[m-n+S], bi[b,m], bj[b,n], ci[b,m], cj[b,n])  (96)
    h1 = relu(e @ w1[:, :, h])     (96->32, per head)
    h2 = relu(h1 @ w2[:, :, h])    (32->16)
    score[b,h,m,n] = h2 @ w3[:, h] (16->1)

Key factorization: layer 1 is linear in the concat, so
    h1pre[b,m,n,h,k] = Ap[m-n+S,h,k] + Arow[b,m,h,k] + Acol[b,n,h,k]
where Ap/Arow/Acol are tiny per-table transforms (computed on-device from the
raw embedding tables and w1).  With the free axis taken as n' = 255-n, the
relative-position gather Ap[m-n+S] becomes a contiguous slice of a 384-wide
table, so no gather is needed at all.  Only layers 2+3 touch the full
(B,S,S,H) volume.

v2 engine plan (per core):
  - DVE: sliding-window add tmp = Ap[m+n'] + Acol (tensor_tensor, 2x mode)
    plus a share of the per-m bias+relu tensor_scalar ops (4x mode).
  - ACT: h2 = relu(ps2) PSUM evacuation + a share of per-m bias+relu.
  - GPSIMD: a share of per-m bias+relu + the fp16 score evacuation.
  - PE: two persistent stationaries (w2f for stage 2, w3stack for stage 3),
    column-tiled so stage-2's two 64-wide group matmuls and stage-3's four
    32-wide variant matmuls run concurrently in disjoint column groups.
  - One fat [128, 512] output DMA per m-block (16 total).

Sharding: 8 cores = batch b (4) x query-half m (2), single SPMD program.
Host does layout only (transpose/reshape/zero-pad/one-hot relabeling).
"""

import os
from contextlib import ExitStack

import numpy as np

import concourse.bacc as bacc
import concourse.bass as bass
import concourse.tile as tile
from concourse import mybir
from concourse.bass_utils import run_bass_kernel_spmd

S = 256
H = 8
B = 4
MH = 128          # m-values per core
VB = 11           # e_bi / e_bj rows  (N_MB + 1)
VC = 102          # e_ci / e_cj rows  (N_C + 2)
APW = MH + S      # 384: width of the per-core shifted e_pos slice
NIT = MH // 8     # 16 m-blocks of 8

F32 = mybir.dt.float32
F16 = mybir.dt.float16
BF16 = mybir.dt.bfloat16

# per-it routing of the 16 per-m bias+relu ops (index s = 8*g + j).
# 'd' = DVE tensor_scalar (fast), 'a' = ACT activation, 'g' = GPSIMD.
TS_ROUTE = os.environ.get("BASSK_TS_ROUTE", "daddaddadaddadda")
H2_ROUTE = os.environ.get("BASSK_H2_ROUTE", "aa")      # per half: a/d/g
SC_ROUTE = os.environ.get("BASSK_SC_ROUTE", "d")       # score evac: g/d/a
N_WARMUP_MM = int(os.environ.get("BASSK_WARMUP", "10"))
NO_TILEPOS = bool(os.environ.get("BASSK_NO_TILEPOS"))  # drop tile_position args
S3_MODE = os.environ.get("BASSK_S3", "tile")           # stage3: tile | acc
TT_ROUTE = os.environ.get("BASSK_TT_ROUTE", "dd")      # per (it%len, g): d/g
TT_MERGE = os.environ.get("BASSK_TT_MERGE", "0") == "1"  # one [128,4096] TT/it


def _declare_io(nc):
    def inp(name, shape):
        return nc.dram_tensor(name, list(shape), F16, kind="ExternalInput").ap()

    ins = {
        "eposT": inp("eposT", (2, 128, APW)),       # [g][hh*32+d, r_local]
        "ebiT": inp("ebiT", (128, VB)),             # [h*16+d, v]
        "ebjT": inp("ebjT", (128, VB)),
        "eciT": inp("eciT", (128, VC)),
        "ecjT": inp("ecjT", (128, VC)),
        "w1pe_blk": inp("w1pe_blk", (2, 128, 128)),  # [g][hh*32+d, hh*32+k]
        "w1bi_blk": inp("w1bi_blk", (128, 256)),     # [h*16+d, h*32+k]
        "w1bj_blk": inp("w1bj_blk", (128, 256)),
        "w1ci_blk": inp("w1ci_blk", (128, 256)),
        "w1cj_blk": inp("w1cj_blk", (128, 256)),
        "oh_b_row": inp("oh_b_row", (VB, MH)),
        "oh_c_row": inp("oh_c_row", (VC, MH)),
        "oh_b_col": inp("oh_b_col", (VB, S)),        # n-reversed
        "oh_c_col": inp("oh_c_col", (VC, S)),
        "w2blk": inp("w2blk", (2, 128, 64)),         # [g][hh*32+k, hh*16+l]
        # stage-3 stationary: col 32q+h = w3[:, h] at rows [64g+16hh+l]
        "w3stack": inp("w3stack", (128, 128)),
        # zero-padded fallback variants (baseline accumulate scheme)
        "w3blk4": inp("w3blk4", (128, 512)),
    }
    # raw per-it PSUM dumps; host unpacks (rows 32q+h, h<8 are live)
    out = nc.dram_tensor("score_part", [NIT, 128, 512], F16,
                         kind="ExternalOutput").ap()
    return ins, out


def _emit(tc: tile.TileContext, X, out):
    nc = tc.nc
    AL = mybir.AluOpType
    AF = mybir.ActivationFunctionType

    with ExitStack() as ctx:
        const = ctx.enter_context(tc.tile_pool(name="const", bufs=1))
        tabs = ctx.enter_context(tc.tile_pool(name="tabs", bufs=1))
        psum_pre = ctx.enter_context(tc.tile_pool(name="psum_pre", bufs=2, space="PSUM"))

        # ---- load raw inputs to SBUF (fp16, two HWDGE issue queues) ----
        ld_n = [0]

        def load(name, src=None):
            if src is None:
                src = X[name]
            t = const.tile(list(src.shape), F16, name=f"sb_{name}")
            eng = nc.sync if ld_n[0] % 2 == 0 else nc.scalar
            ld_n[0] += 1
            eng.dma_start(out=t, in_=src)
            return t

        # ---- PE warm-up: dummy matmuls so the HAM clock-gate opens before
        # the precompute/main matmuls start; overlaps the input-DMA phase.
        warm_w = const.tile([128, 128], BF16, name="warm_w")
        warm_r = const.tile([128, 512], BF16, name="warm_r")
        nc.vector.memset(warm_w, 0.0)
        nc.vector.memset(warm_r, 0.0)
        ps_warm = psum_pre.tile([128, 512], F32, name="ps_warm", tag="pre")
        for _ in range(N_WARMUP_MM):
            nc.tensor.matmul(out=ps_warm, lhsT=warm_w, rhs=warm_r,
                             start=True, stop=True)

        # critical-path loads first: Ap chain, then Acol chain, then the rest
        eposT = [load(f"eposT{g}", X["eposT"][g]) for g in range(2)]
        w1pe = [load(f"w1pe{g}", X["w1pe_blk"][g]) for g in range(2)]
        ebjT = load("ebjT")
        ecjT = load("ecjT")
        w1bj = load("w1bj_blk")
        w1cj = load("w1cj_blk")
        ohbc = load("oh_b_col")
        ohcc = load("oh_c_col")
        ebiT = load("ebiT")
        eciT = load("eciT")
        w1bi = load("w1bi_blk")
        w1ci = load("w1ci_blk")
        ohbr = load("oh_b_row")
        ohcr = load("oh_c_row")
        w2h = [load(f"w2h_{g}", X["w2blk"][g]) for g in range(2)]
        w2f = tabs.tile([128, 128], F16, name="w2f")
        for g in range(2):
            nc.vector.tensor_copy(w2f[:, 64 * g:64 * (g + 1)], w2h[g])
        w3stack = load("w3stack")
        w3f = load("w3blk4") if S3_MODE == "acc" else None

        # ---- A-tables (all fp32 matmuls, accumulated in PSUM) ----
        # Ap[g]: [128=(hh,k), APW+4] fp16, col j <-> r_local = j-2
        Ap = [tabs.tile([128, APW + 4], F16, name=f"Ap{g}") for g in range(2)]
        for g in range(2):
            ps = psum_pre.tile([128, APW - 1], F32, name=f"ps_ap{g}", tag="pre")
            nc.tensor.matmul(out=ps, lhsT=w1pe[g], rhs=eposT[g][:, 1:APW],
                             start=True, stop=True)
            nc.vector.tensor_copy(Ap[g][:, 2:APW + 1], ps)

        # T-tables: [v, (h,k)=256] fp16
        T = {}
        for nm, eT, wblk, P in (("bi", ebiT, w1bi, VB), ("ci", eciT, w1ci, VC),
                                ("bj", ebjT, w1bj, VB), ("cj", ecjT, w1cj, VC)):
            ps = psum_pre.tile([P, 256], F32, name=f"ps_t{nm}", tag="pre")
            nc.tensor.matmul(out=ps, lhsT=eT, rhs=wblk, start=True, stop=True)
            t = tabs.tile([P, 256], F16, name=f"T{nm}")
            nc.vector.tensor_copy(t, ps)
            T[nm] = t

        # Arow[g]: [128=(hh,k), MH] fp32 ; Acol8[g]: [128, 8*S] fp16 (x8 copies)
        Arow = [tabs.tile([128, MH], F32, name=f"Arow{g}") for g in range(2)]
        Acol8 = [tabs.tile([128, 8 * S], F16, name=f"Acol8_{g}") for g in range(2)]
        for g in range(2):
            cs = slice(128 * g, 128 * (g + 1))
            ps = psum_pre.tile([128, MH], F32, name=f"ps_arow{g}", tag="pre")
            nc.tensor.matmul(out=ps, lhsT=T["bi"][:, cs], rhs=ohbr,
                             start=True, stop=False)
            nc.tensor.matmul(out=ps, lhsT=T["ci"][:, cs], rhs=ohcr,
                             start=False, stop=True)
            nc.vector.tensor_copy(Arow[g], ps)

            ps2 = psum_pre.tile([128, S], F32, name=f"ps_acol{g}", tag="pre")
            nc.tensor.matmul(out=ps2, lhsT=T["bj"][:, cs], rhs=ohbc,
                             start=True, stop=False)
            nc.tensor.matmul(out=ps2, lhsT=T["cj"][:, cs], rhs=ohcc,
                             start=False, stop=True)
            nc.vector.tensor_copy(Acol8[g][:, 0:S], ps2)
            for w in (S, 2 * S, 4 * S):
                nc.vector.tensor_copy(Acol8[g][:, w:2 * w], Acol8[g][:, 0:w])

        # ---- main loop: 16 m-blocks x 8 m-values ----
        work = ctx.enter_context(tc.tile_pool(name="work", bufs=3))
        psum_m = ctx.enter_context(tc.tile_pool(name="psum_m", bufs=1, space="PSUM"))
        psum_o = ctx.enter_context(tc.tile_pool(name="psum_o", bufs=2, space="PSUM"))

        def bias_relu(dst, src, bias_col, eng):
            if eng == "d":
                nc.vector.tensor_scalar(dst, src, bias_col, 0.0, AL.add, AL.max)
            elif eng == "g":
                nc.gpsimd.tensor_scalar(dst, src, bias_col, 0.0, AL.add, AL.max)
            else:
                nc.scalar.activation(out=dst, in_=src, func=AF.Relu,
                                     bias=bias_col, scale=1.0)

        for it in range(NIT):
            m0 = 8 * it
            # -- stage 1a: tmp[g] = Ap[g][:, m+n'+2] + Acol (sliding window)
            if True:
                tmps = []
                for g in range(2):
                    apg = Ap[g]
                    apwin = bass.AP(
                        apg.tensor, apg.offset + (m0 + 2),
                        [list(apg.ap[0]), [1, 8], [1, S]],
                    )
                    tmp = work.tile([128, 2048], F16, name=f"tmp{g}_{it}",
                                    tag=f"tmp{g}")
                    if TT_ROUTE[(2 * it + g) % len(TT_ROUTE)] == "g":
                        nc.gpsimd.tensor_add(tmp, apwin, Acol8[g])
                    else:
                        nc.vector.tensor_add(tmp, apwin, Acol8[g])
                    tmps.append(tmp)

            # -- stage 1b: h1 = relu(tmp + Arow[:, m]) per m, routed 3 ways
            h1 = [[work.tile([128, 1024], F16, name=f"h1_{g}_{it}_{hf}",
                             tag=f"h1{g}{hf}") for hf in range(2)]
                  for g in range(2)]
            for g in range(2):
                for j in range(8):
                    hf, jj = j // 4, j % 4
                    bias_relu(h1[g][hf][:, S * jj:S * (jj + 1)],
                              tmps[g][:, S * j:S * (j + 1)],
                              Arow[g][:, m0 + j:m0 + j + 1],
                              TS_ROUTE[(8 * g + j) % len(TS_ROUTE)])

            # -- stage 2: both halves, two col-tiled group matmuls each
            ps2h = []
            for hf in range(2):
                ps2 = psum_m.tile([128, 1024], F32, name=f"ps2_{it}_{hf}",
                                  tag=f"ps2{hf}")
                for c in range(2):
                    for g in range(2):
                        nc.tensor.matmul(
                            out=ps2[64 * g:64 * (g + 1), 512 * c:512 * (c + 1)],
                            lhsT=w2f[:, 64 * g:64 * (g + 1)],
                            rhs=h1[g][hf][:, 512 * c:512 * (c + 1)],
                            start=True, stop=True,
                            tile_position=None if NO_TILEPOS else (0, 64 * g),
                        )
                ps2h.append(ps2)

            # -- h2 = relu(ps2): PSUM -> SBUF fp16
            h2 = []
            for hf in range(2):
                t = work.tile([128, 1024], F16, name=f"h2_{it}_{hf}", tag=f"h2{hf}")
                r = H2_ROUTE[hf % len(H2_ROUTE)]
                if r == "a":
                    nc.scalar.activation(out=t, in_=ps2h[hf], func=AF.Relu)
                elif r == "d":
                    nc.vector.tensor_scalar(t, ps2h[hf], 0.0, 0.0, AL.max, AL.bypass)
                else:
                    nc.gpsimd.tensor_scalar(t, ps2h[hf], 0.0, 0.0, AL.max, AL.bypass)
                h2.append(t)

            # -- stage 3: four col-tiled variant matmuls into one PSUM bank
            ps3 = psum_o.tile([128, 512], F32, name=f"ps3_{it}", tag="ps3")
            for hf in range(2):
                for c in range(2):
                    q = 2 * hf + c
                    if S3_MODE == "acc":
                        nc.tensor.matmul(
                            out=ps3, lhsT=w3f[:, 128 * q:128 * (q + 1)],
                            rhs=h2[hf][:, 512 * c:512 * (c + 1)],
                            start=(q == 0), stop=(q == 3),
                        )
                    else:
                        nc.tensor.matmul(
                            out=ps3[32 * q:32 * (q + 1), :],
                            lhsT=w3stack[:, 32 * q:32 * (q + 1)],
                            rhs=h2[hf][:, 512 * c:512 * (c + 1)],
                            start=True, stop=True,
                            tile_position=None if NO_TILEPOS else (0, 32 * q),
                        )

            # -- score evac (fp16) + one fat DMA out
            sc = work.tile([128, 512], F16, name=f"sc_{it}", tag="sc")
            if SC_ROUTE == "g":
                nc.gpsimd.tensor_copy(sc, ps3)
            elif SC_ROUTE == "d":
                nc.vector.tensor_copy(sc, ps3)
            else:
                nc.scalar.copy(sc, ps3)
            nc.sync.dma_start(out=out[it], in_=sc)


_PROGRAM = None


def _get_program():
    global _PROGRAM
    if _PROGRAM is None:
        nc = bacc.Bacc("TRN2", debug=False, num_devices=8)
        ins, out = _declare_io(nc)
        with tile.TileContext(nc) as tc:
            _emit(tc, ins, out)
        nc.compile()
        _PROGRAM = nc
    return _PROGRAM


def _build_in_maps(inputs):
    b_seq = np.asarray(inputs["b_seq"]).astype(np.int64)
    c_seq = np.asarray(inputs["c_seq"]).astype(np.int64)
    e_pos = np.asarray(inputs["e_pos"]).astype(np.float32)   # (512, 8, 32)
    e_bi = np.asarray(inputs["e_bi"]).astype(np.float32)     # (11, 8, 16)
    e_bj = np.asarray(inputs["e_bj"]).astype(np.float32)
    e_ci = np.asarray(inputs["e_ci"]).astype(np.float32)     # (102, 8, 16)
    e_cj = np.asarray(inputs["e_cj"]).astype(np.float32)
    w1 = np.asarray(inputs["w1_e"]).astype(np.float32)       # (96, 32, 8)
    w2 = np.asarray(inputs["w2_e"]).astype(np.float32)       # (32, 16, 8)
    w3 = np.asarray(inputs["w3_e"]).astype(np.float32)       # (16, 8)

    C = lambda a: np.ascontiguousarray(a.astype(np.float16))

    # [h*32+d, r] and [h*16+d, v] transposed table layouts
    eposT_full = C(e_pos.transpose(1, 2, 0).reshape(H * 32, 2 * S))
    ebiT = C(e_bi.transpose(1, 2, 0).reshape(128, VB))
    ebjT = C(e_bj.transpose(1, 2, 0).reshape(128, VB))
    eciT = C(e_ci.transpose(1, 2, 0).reshape(128, VC))
    ecjT = C(e_cj.transpose(1, 2, 0).reshape(128, VC))

    # block-diagonal w1 pieces
    w1pe_blk = np.zeros((2, 128, 128), np.float16)
    for g in range(2):
        for hh in range(4):
            w1pe_blk[g, 32 * hh:32 * (hh + 1), 32 * hh:32 * (hh + 1)] = w1[0:32, :, 4 * g + hh]

    def blk16(w1rows):  # (16, 32, 8) -> [h*16+d, h*32+k]
        m = np.zeros((128, 256), np.float16)
        for h in range(H):
            m[16 * h:16 * (h + 1), 32 * h:32 * (h + 1)] = w1rows[:, :, h]
        return m

    w1bi_blk = blk16(w1[32:48])
    w1bj_blk = blk16(w1[48:64])
    w1ci_blk = blk16(w1[64:80])
    w1cj_blk = blk16(w1[80:96])

    w2blk = np.zeros((2, 128, 64), np.float16)
    for g in range(2):
        for hh in range(4):
            w2blk[g, 32 * hh:32 * (hh + 1), 16 * hh:16 * (hh + 1)] = w2[:, :, 4 * g + hh]

    # stage-3 stationary: rows (g,hh,l) = h2 partition layout; col 32q+h
    w3stack = np.zeros((128, 128), np.float16)
    w3blk4 = np.zeros((128, 512), np.float16)
    for q in range(4):
        for h in range(H):
            w3stack[16 * h:16 * (h + 1), 32 * q + h] = w3[:, h]
            w3blk4[16 * h:16 * (h + 1), 128 * q + 32 * q + h] = w3[:, h]

    shared = {
        "ebiT": ebiT, "ebjT": ebjT, "eciT": eciT, "ecjT": ecjT,
        "w1pe_blk": w1pe_blk, "w1bi_blk": w1bi_blk, "w1bj_blk": w1bj_blk,
        "w1ci_blk": w1ci_blk, "w1cj_blk": w1cj_blk,
        "w2blk": w2blk, "w3stack": w3stack, "w3blk4": w3blk4,
    }

    def onehot(seq_slice, nv):
        oh = np.zeros((nv, len(seq_slice)), np.float16)
        oh[seq_slice, np.arange(len(seq_slice))] = 1.0
        return oh

    in_maps = []
    for core in range(8):
        b, half = core // 2, core % 2
        m_off = half * MH
        im = dict(shared)
        im["eposT"] = C(eposT_full[:, m_off:m_off + APW].reshape(2, 128, APW))
        im["oh_b_row"] = onehot(b_seq[b, m_off:m_off + MH], VB)
        im["oh_c_row"] = onehot(c_seq[b, m_off:m_off + MH], VC)
        im["oh_b_col"] = onehot(b_seq[b, ::-1], VB)
        im["oh_c_col"] = onehot(c_seq[b, ::-1], VC)
        in_maps.append(im)
    return in_maps


def _unpack_part(part):
    """[NIT, 128, 512] fp16 raw dump -> [H, MH, S] fp32 (n-unreversed)."""
    arr = np.asarray(part, np.float32).reshape(NIT, 4, 32, 2, S)
    arr = arr[:, :, :H]                      # live head rows    [it,q,h,u,n']
    arr = arr.transpose(2, 0, 1, 3, 4)       # [h, it, q, u, n']
    arr = arr.reshape(H, MH, S)[:, :, ::-1]  # m = 8*it + 2*q + u; un-reverse n
    return arr


def _assemble(core_outs):
    score = np.empty((B, H, S, S), np.float32)
    for core in range(8):
        b, half = core // 2, core % 2
        score[b, :, half * MH:(half + 1) * MH, :] = _unpack_part(
            core_outs[core]["score_part"])
    return score


def kernel(**inputs) -> np.ndarray:
    in_maps = _build_in_maps(inputs)
    nc = _get_program()

    if os.environ.get("BASSK_SIM"):
        from concourse.bass_interp import CoreSim
        score = np.zeros((B, H, S, S), np.float32)
        for core in [int(x) for x in os.environ["BASSK_SIM"].split(",")]:
            sim = CoreSim(nc, trace=False)
            for k, v in in_maps[core].items():
                sim.tensor(k)[:] = v
            sim.simulate(check_with_hw=False)
            b, half = core // 2, core % 2
            score[b, :, half * MH:(half + 1) * MH, :] = _unpack_part(
                sim.tensor("score_part").copy())
        return score

    res = run_bass_kernel_spmd(nc, in_maps, core_ids=list(range(8)))
    return _assemble(res.results)
